# revision 33
# baseline (speedup 1.0000x reference)
"""Causal self-attention (B=8, T=1024, C=768, H=12) on 8 Trainium2 NeuronCores.

Sharding: data parallel - one batch element per core, no collectives.

All matmul operands are bf16 (rel err ~4e-3 vs the fp32 reference, 5x
margin on the 2e-2 budget): same 1 cyc/row as fp32r on the PE but half the
DMA bytes and no <256-wide 4 cyc/row penalties. The schedule keeps the PE
(the bottleneck engine, ~111 us busy/body) fed while the softmax exp
(ACT, ~5.4 us/head) drains:

  warmup matmuls on a memset tile cover the initial DMA wait and keep the
    PE p-state clock ramped (gaps >100 ns drop the clock to half speed)
  QKV feature-tile chains are interleaved INTO the attention head loop:
    pair j computes S/O for heads (2j+1, 2j) while the chains for feature
    tile j+1 fill the PE between S and O; the last pair (no chains left)
    gets warmup filler instead
  causal mask of each diagonal block is a 0/1 lower-triangle multiply on
    the DVE (4x bf16 mode, ~127 ns) instead of a PE mask matmul
  every head computes O with lhsT = [1|V]: the softmax denominator lands
    on PSUM partition 0 where gpsimd partition_broadcast reads it directly;
    the normalized O goes to its attnT partition slot via a small
    SBUF->SBUF DMA (DVE lanes are partition-locked)
  O/chain/V PSUM tiles share one 4-slot pool and are freed ~0.7 us after
    the matmul by an immediate PSUM->SBUF copy (DVE/ACT alternating), so
    the slow normalize chain never holds a PSUM bank
  the last head runs its query chunks high-to-low and projection t-tiles
    run 4..7 first, so the projection tail overlaps the final normalize
  DMAs are consolidated (one per x/weight column-group, weights
    host-permuted into consumption order) to respect the ~565 ns/issue SP
    sequencer cost

Measured (async-pipelined steady state, 8 cores): ~208 us/body vs ~280 us
for the fp32r baseline (~26% faster); rel err 3.95e-3.
"""
import sys
from contextlib import ExitStack

import numpy as np

for _p in ("/opt/trn_rl_repo", "/root/.axon_site/_ro/trn_rl_repo"):
    if _p not in sys.path:
        sys.path.insert(0, _p)

import concourse.bass as bass  # noqa: E402
import concourse.mybir as mybir  # noqa: E402

F32 = mybir.dt.float32
BF16 = mybir.dt.bfloat16
AF = mybir.ActivationFunctionType
OP = mybir.AluOpType

B, T, C, H, D = 8, 1024, 768, 12, 64
N_CORES = 8
NT = T // 128  # 8
NC = C // 128  # 6
N_WARMUP = 32  # matmuls covering DMA lead-in (~110 ns each once ramped)


def _chunks_512(a, b):
    out = []
    while a < b:
        nxt = min((a // 512 + 1) * 512, b)
        out.append((a, nxt))
        a = nxt
    return out


def _emit_attention(tc, io):
    nc = tc.nc

    off = [0] * (NT + 1)
    for i in range(NT):
        off[i + 1] = off[i] + (T - 128 * i)
    PTW = off[NT]

    with ExitStack() as stack:
        consts = stack.enter_context(tc.tile_pool(name="consts", bufs=1))
        persist = stack.enter_context(tc.tile_pool(name="persist", bufs=1))

        c16 = consts.tile([128, 2, 128], BF16, tag="c16")  # tri01 | ones
        tri01 = c16[:, 0, :]
        ones_sb = c16[:, 1, :]
        cf = consts.tile([128, 2 * NC + 2 * C], F32, tag="cf")  # bqt | bb
        bqt_sb = cf[:, 0:2 * NC]
        bb_sb = cf[:, 2 * NC:]

        qt = persist.tile([128, NC, T], BF16, tag="qt")
        kt_ = persist.tile([128, NC, T], BF16, tag="kt")
        vp = persist.tile([128, NT, H, D + 1], BF16, tag="vp")

        p1w = stack.enter_context(tc.tile_pool(name="p1w", bufs=1))
        x1t = p1w.tile([128, NC, T], BF16, tag="x1t")
        wq_sb = p1w.tile([128, NC, 3 * C], BF16, tag="wq")

        p23 = stack.enter_context(tc.tile_pool(name="p23", bufs=1))
        attnT = p23.tile([128, NC, T], BF16, tag="attnT")
        wpp = p23.tile([128, NC, C], BF16, tag="wpp")

        # ---- warmup: keep the PE busy + p-state ramping during DMA lead-in
        warm = consts.tile([128, 128], BF16, tag="warm")
        with tc.tile_pool(name="ps0", bufs=1, space="PSUM") as ps0:
            nc.vector.memzero(warm[:])
            wps = ps0.tile([128, 128], F32, tag="wps")
            for _ in range(N_WARMUP):
                nc.tensor.matmul(wps[:], warm[:], warm[:], start=True, stop=True)

        # ---- DMA issue order (SP sequencer ~565 ns per dma_start).
        # wqkv16 is host-permuted into consumption order: the kernel's
        # 128-col block j of wq_sb holds original feature block WPERM[j].
        def dma_wcols(w0, w1):
            nc.sync.dma_start(
                wq_sb[:, :, w0:w1],
                io["wqkv16"][:, w0:w1].rearrange("(k p) w -> p k w", p=128))

        nc.sync.dma_start(x1t[:, 0, :], io["xT16"][0:128, :])
        nc.sync.dma_start(wq_sb[:, 0, 0:256], io["wqkv16"][0:128, 0:256])
        nc.sync.dma_start(
            x1t[:, 1:3, :],
            io["xT16"][128:384, :].rearrange("(k p) t -> p k t", p=128))
        nc.sync.dma_start(          # Q0 | K0 for kt 1..5
            wq_sb[:, 1:6, 0:256],
            io["wqkv16"][128:768, 0:256].rearrange("(k p) w -> p k w", p=128))
        nc.sync.dma_start(
            x1t[:, 3:6, :],
            io["xT16"][384:768, :].rearrange("(k p) t -> p k t", p=128))
        nc.sync.dma_start(c16[:], io["consts16"].rearrange("p (g w) -> p g w", g=2))
        nc.sync.dma_start(cf[:], io["constsf"])
        dma_wcols(256, 512)          # Q1 | K1
        dma_wcols(512, 1280)         # V
        dma_wcols(1280, 1792)        # Q2 K2 Q3 K3
        dma_wcols(1792, 2304)        # Q4 K4 Q5 K5
        nc.sync.dma_start(wpp[:],
                          io["wp16"].rearrange("(k p) w -> p k w", p=128))

        # ones column 0 of vp ([1|V] puts the softmax denominator on row 0)
        nc.vector.tensor_copy(
            vp[:, :, :, 0],
            ones_sb[:, 0:NT * H].rearrange("p (t h) -> p t h", h=H))

        # PSUM budget (8 banks of 512 f32): psv 4 (chains+V+O share) + ps_s 2x2
        stack12 = stack.enter_context(ExitStack())
        psv = stack12.enter_context(tc.tile_pool(name="psv", bufs=4, space="PSUM"))
        ps2a = stack12.enter_context(tc.tile_pool(name="ps2a", bufs=2, space="PSUM"))
        p2 = stack12.enter_context(tc.tile_pool(name="p2", bufs=3))
        p2o = stack12.enter_context(tc.tile_pool(name="p2o", bufs=5))

        # wq_sb 128-col block position of original feature block m
        # (host layout: Q0 K0 Q1 K1 V0..V5 Q2 K2 Q3 K3 Q4 K4 Q5 K5)
        WPOS = {0: 0, 6: 1, 1: 2, 7: 3, 2: 10, 8: 11, 3: 12, 9: 13,
                4: 14, 10: 15, 5: 16, 11: 17}

        def emit_chain(m):
            """One 128-col feature tile of Q^T (m<NC) or K^T (m>=NC)."""
            dest = qt if m < NC else kt_
            mm = m % NC
            wc = WPOS[m] * 128
            for (a, b) in _chunks_512(0, T):
                ps = psv.tile([128, 512], F32, tag="psv")
                for kt in range(NC):
                    nc.tensor.matmul(
                        ps[:, 0:b - a], wq_sb[:, kt, wc:wc + 128],
                        x1t[:, kt, a:b], start=(kt == 0), stop=(kt == NC - 1))
                nc.vector.tensor_scalar_add(dest[:, mm, a:b], ps[:, 0:b - a],
                                            bqt_sb[:, m:m + 1])

        def emit_V():
            for t in range(NT):
                for (n0, n1) in _chunks_512(0, C):
                    ps = psv.tile([128, 512], F32, tag="psv")
                    for kt in range(NC):
                        nc.tensor.matmul(
                            ps[:, 0:n1 - n0], x1t[:, kt, t * 128:(t + 1) * 128],
                            wq_sb[:, kt, 512 + n0:512 + n1],
                            start=(kt == 0), stop=(kt == NC - 1))
                    h0, h1 = n0 // D, n1 // D
                    nc.vector.tensor_tensor(
                        vp[:, t, h0:h1, 1:D + 1],
                        ps[:, 0:n1 - n0].rearrange("p (h d) -> p h d", d=D),
                        bb_sb[:, n0:n1].rearrange("p (h d) -> p h d", d=D),
                        OP.add)

        pt_tiles = {}

        def emit_S(h):
            p0 = 64 * (h % 2)
            ft = h // 2
            # per-strip tiles: fine-grained deps
            pts = [p2.tile([128, T - 128 * kt], BF16, tag=f"pt{kt}",
                           name=f"pt{h}_{kt}") for kt in range(NT)]
            pt_tiles[h] = pts
            for kt in range(NT):
                base = (kt * 128 // 512) * 512
                ps_s = ps2a.tile([128, T - base], F32, tag="ps_s")
                for (a, b) in _chunks_512(kt * 128, T):
                    nc.tensor.matmul(
                        ps_s[:, a - base:b - base],
                        kt_[p0:p0 + 64, ft, kt * 128:(kt + 1) * 128],
                        qt[p0:p0 + 64, ft, a:b],
                        start=True, stop=True)
                nc.scalar.activation(
                    pts[kt][:, :],
                    ps_s[:, kt * 128 - base:T - base],
                    AF.Exp, bias=0.0, scale=1.0 / np.sqrt(D))
                # causal mask of the diagonal block: multiply by the 0/1
                # lower-triangle tile (DVE 4x bf16 mode) instead of a PE
                # mask matmul
                nc.vector.tensor_tensor(
                    pts[kt][:, 0:128], pts[kt][:, 0:128],
                    tri01[:, :], OP.mult)

        def emit_O(h, rev=False):
            p0 = 64 * (h % 2)
            ft = h // 2
            pts = pt_tiles.pop(h)
            qchunks = _chunks_512(0, T)
            if rev:
                qchunks = qchunks[::-1]
            tail = []
            for ci, (q0, q1) in enumerate(qchunks):
                kt_max = q1 // 128
                w = q1 - q0
                ps_o = psv.tile([65, 512], F32, tag="psv")
                for kt in range(kt_max):
                    a = max(q0, kt * 128)
                    rhs = pts[kt][:, a - kt * 128:q1 - kt * 128]
                    nc.tensor.matmul(
                        ps_o[:, a - q0:q1 - q0],
                        vp[:, kt, h, :], rhs,
                        start=(kt == 0), stop=(kt == kt_max - 1))
                # copy to SBUF right away: frees the PSUM bank in ~0.7 us
                # instead of holding it through the whole normalize chain
                o_sb = p2o.tile([65, 512], F32, tag="o_sb")
                if h % 2:
                    nc.vector.tensor_copy(o_sb[:, 0:w], ps_o[:, 0:w])
                else:
                    nc.scalar.copy(o_sb[:, 0:w], ps_o[:, 0:w])  # ACT has slack
                # [1|V] puts the denominator on row 0: partition_broadcast
                # reads it directly
                dn = p2o.tile([1, 512], F32, tag="dn")
                bc = p2o.tile([128, 512], F32, tag="bc")
                nc.vector.reciprocal(dn[0:1, 0:w], o_sb[0:1, 0:w])
                nc.gpsimd.partition_broadcast(bc[:, 0:w], dn[0:1, 0:w])
                o_n = p2o.tile([65, 512], BF16, tag="o_n")
                tail.append((o_sb, bc, o_n, q0, q1, w))
                if ci == 0:
                    continue
                _flush_norm(tail.pop(0), ft, p0)
            for item in tail:
                _flush_norm(item, ft, p0)

        def _flush_norm(item, ft, p0):
            o_sb, bc, o_n, q0, q1, w = item
            # row 0 computes d*(1/d) - harmless; DVE partition base must be
            # 32-aligned so the O rows (1..65) can't be addressed directly
            nc.vector.tensor_tensor(o_n[0:65, 0:w], o_sb[0:65, 0:w],
                                    bc[0:65, 0:w], OP.mult)
            # DVE lanes are partition-locked; route the normalized O to its
            # head's partition slot via a small SBUF->SBUF DMA
            nc.sync.dma_start(attnT[p0:p0 + 64, ft, q0:q1], o_n[1:65, 0:w])

        # ---- master schedule -------------------------------------------
        emit_chain(0)      # Q feature tile 0
        emit_chain(NC)     # K feature tile 0
        emit_S(1)
        emit_S(0)
        emit_chain(1)
        emit_chain(NC + 1)
        emit_V()
        emit_O(1)
        emit_O(0)
        def warm_fill(n):
            # keep the PE busy (and its p-state clock hot) through a known
            # stall; the tiles' only consumer is the PE itself
            for _ in range(n):
                wt = psv.tile([128, 128], F32, tag="psv", name="wfill")
                nc.tensor.matmul(wt[:, 0:128], warm[:], warm[:],
                                 start=True, stop=True)

        for j in range(1, H // 2):
            emit_S(2 * j + 1)
            emit_S(2 * j)
            if j + 1 < H // 2:
                # fill the PE while exp(2j+1) drains on ACT; PSUM banks free
                # fast (the copy is the only PSUM reader in the norm chain)
                emit_chain(j + 1)
                emit_chain(NC + j + 1)
            else:
                warm_fill(20)  # no chains left: cover the exp(11) wait
            emit_O(2 * j + 1)
            emit_O(2 * j, rev=(j == H // 2 - 1))

        stack12.close()  # free phase-1/2 PSUM banks for the projection

        # ---------------- projection ----------------
        with tc.tile_pool(name="p3", bufs=3) as p3, \
             tc.tile_pool(name="ps3", bufs=2, space="PSUM") as ps3:
            # last head finished queries 512.. first -> those t-tiles first
            for t in [4, 5, 6, 7, 0, 1, 2, 3]:
                for (n0, n1) in _chunks_512(0, C):
                    ps_y = ps3.tile([128, 512], F32, tag="ps_y")
                    for kt in range(NC):
                        nc.tensor.matmul(
                            ps_y[:, 0:n1 - n0],
                            attnT[:, kt, t * 128:(t + 1) * 128],
                            wpp[:, kt, n0:n1],
                            start=(kt == 0), stop=(kt == NC - 1))
                    y_sb = p3.tile([128, 512], F32, tag="y_sb")
                    nc.vector.tensor_tensor(y_sb[:, 0:n1 - n0],
                                            ps_y[:, 0:n1 - n0],
                                            bb_sb[:, C + n0:C + n1], OP.add)
                    nc.sync.dma_start(io["y"][t * 128:(t + 1) * 128, n0:n1],
                                      y_sb[:, 0:n1 - n0])


def declare_io(nc):
    return {
        "xT16": nc.dram_tensor("xT16", [C, T], BF16, kind="ExternalInput").ap(),
        "wqkv16": nc.dram_tensor("wqkv16", [C, 3 * C], BF16,
                                 kind="ExternalInput").ap(),
        "wp16": nc.dram_tensor("wp16", [C, C], BF16, kind="ExternalInput").ap(),
        "consts16": nc.dram_tensor("consts16", [128, 2 * 128], BF16,
                                   kind="ExternalInput").ap(),
        "constsf": nc.dram_tensor("constsf", [128, 2 * NC + 2 * C], F32,
                                  kind="ExternalInput").ap(),
        "y": nc.dram_tensor("y", [T, C], F32, kind="ExternalOutput").ap(),
    }


def build_nc():
    from concourse import bacc
    import concourse.tile as tile
    nc = bacc.Bacc("TRN2", target_bir_lowering=False, debug=False,
                   enable_asserts=True, num_devices=N_CORES)
    io = declare_io(nc)
    with tile.TileContext(nc) as tc:
        _emit_attention(tc, io)
    nc.compile()
    return nc


def host_consts():
    import ml_dtypes
    # tri01[key p, query c] = 1 where c >= p (keep), 0 above the diagonal
    tri01 = np.triu(np.ones((128, 128), dtype=np.float32))
    consts16 = np.concatenate([
        tri01.astype(ml_dtypes.bfloat16),
        np.ones((128, 128), dtype=ml_dtypes.bfloat16),
    ], axis=1)
    return {"consts16": np.ascontiguousarray(consts16)}


_NC_CACHE = None


def _get_nc():
    global _NC_CACHE
    if _NC_CACHE is None:
        _NC_CACHE = build_nc()
    return _NC_CACHE


# host wqkv column-block order (orig 128-col block index): Q0 K0 Q1 K1
# V0..V5 Q2 K2 Q3 K3 Q4 K4 Q5 K5 -- matches WPOS in _emit_attention
WPERM = [0, 6, 1, 7, 12, 13, 14, 15, 16, 17, 2, 8, 3, 9, 4, 10, 5, 11]


def make_in_maps(x, c_attn_kernel, c_attn_bias, c_proj_kernel, c_proj_bias):
    import ml_dtypes
    BF = ml_dtypes.bfloat16
    consts = host_consts()
    wq_f32 = np.asarray(c_attn_kernel, dtype=np.float32)
    wqkv16 = np.ascontiguousarray(np.concatenate(
        [wq_f32[:, j * 128:(j + 1) * 128] for j in WPERM], axis=1)).astype(BF)
    bqkv = np.ascontiguousarray(c_attn_bias, dtype=np.float32)
    wp16 = np.ascontiguousarray(c_proj_kernel, dtype=np.float32).astype(BF)
    bp = np.ascontiguousarray(c_proj_bias, dtype=np.float32)
    bqt = np.ascontiguousarray(bqkv[:2 * C].reshape(2 * NC, 128).T)
    bias_bcast = np.tile(np.concatenate([bqkv[2 * C:], bp]), (128, 1))
    constsf = np.ascontiguousarray(
        np.concatenate([bqt, bias_bcast], axis=1), dtype=np.float32)
    in_maps = []
    for bb in range(N_CORES):
        m = {"xT16": np.ascontiguousarray(
                np.asarray(x[bb], np.float32).T).astype(BF),
             "wqkv16": wqkv16, "wp16": wp16, "constsf": constsf}
        m.update(consts)
        in_maps.append(m)
    return in_maps


def kernel(x, c_attn_kernel, c_attn_bias, c_proj_kernel, c_proj_bias):
    from concourse.bass_utils import run_bass_kernel_spmd
    x = np.asarray(x)
    assert x.shape == (B, T, C), x.shape
    nc = _get_nc()
    in_maps = make_in_maps(x, c_attn_kernel, c_attn_bias, c_proj_kernel,
                           c_proj_bias)
    res = run_bass_kernel_spmd(nc, in_maps, core_ids=list(range(N_CORES)))
    y = np.stack([res.results[bb]["y"] for bb in range(N_CORES)]).astype(np.float32)
    return y


# revision 34
# speedup vs baseline: 1.0377x; 1.0377x over previous
"""Causal self-attention (B=8, T=1024, C=768, H=12) on 8 Trainium2 NeuronCores.

Sharding: data parallel - one batch element per core, no collectives.

All matmul operands are bf16 (rel err ~4e-3 vs the fp32 reference, 5x
margin on the 2e-2 budget): same 1 cyc/row as fp32r on the PE but half the
DMA bytes and no <256-wide 4 cyc/row penalties. The schedule keeps the PE
(the bottleneck engine, ~111 us busy/body) fed while the softmax exp
(ACT, ~5.4 us/head) drains:

  warmup matmuls on a memset tile cover the initial DMA wait and keep the
    PE p-state clock ramped (gaps >100 ns drop the clock to half speed)
  QKV feature-tile chains are interleaved INTO the attention head loop:
    pair j computes S/O for heads (2j+1, 2j) while the chains for feature
    tile j+1 fill the PE between S and O; the last pair (no chains left)
    gets warmup filler instead
  causal mask of each diagonal block is a 0/1 lower-triangle multiply on
    the DVE (4x bf16 mode, ~127 ns) instead of a PE mask matmul
  every head computes O with lhsT = [1|V]: the softmax denominator lands
    on PSUM partition 0 where gpsimd partition_broadcast reads it directly;
    the normalized O goes to its attnT partition slot via a small
    SBUF->SBUF DMA (DVE lanes are partition-locked)
  O/chain/V PSUM tiles share one 4-slot pool and are freed ~0.7 us after
    the matmul by an immediate PSUM->SBUF copy (DVE/ACT alternating), so
    the slow normalize chain never holds a PSUM bank
  the last head runs its query chunks high-to-low and projection t-tiles
    run 4..7 first, so the projection tail overlaps the final normalize
  DMAs are consolidated (one per x/weight column-group, weights
    host-permuted into consumption order) to respect the ~565 ns/issue SP
    sequencer cost

  softmax exp runs as 6 ACT ops/head, not 8: key-strips 5/6/7 pack into
    one 2-bank PSUM tile (bank-aligned writes) under a single exp

Measured (async-pipelined steady state, 8 cores): ~206 us/body vs ~280 us
for the fp32r baseline (~26% faster); rel err 3.95e-3.
"""
import sys
from contextlib import ExitStack

import numpy as np

for _p in ("/opt/trn_rl_repo", "/root/.axon_site/_ro/trn_rl_repo"):
    if _p not in sys.path:
        sys.path.insert(0, _p)

import concourse.bass as bass  # noqa: E402
import concourse.mybir as mybir  # noqa: E402

F32 = mybir.dt.float32
BF16 = mybir.dt.bfloat16
AF = mybir.ActivationFunctionType
OP = mybir.AluOpType

B, T, C, H, D = 8, 1024, 768, 12, 64
N_CORES = 8
NT = T // 128  # 8
NC = C // 128  # 6
N_WARMUP = 32  # matmuls covering DMA lead-in (~110 ns each once ramped)


def _chunks_512(a, b):
    out = []
    while a < b:
        nxt = min((a // 512 + 1) * 512, b)
        out.append((a, nxt))
        a = nxt
    return out


def _emit_attention(tc, io):
    nc = tc.nc

    off = [0] * (NT + 1)
    for i in range(NT):
        off[i + 1] = off[i] + (T - 128 * i)
    PTW = off[NT]

    with ExitStack() as stack:
        consts = stack.enter_context(tc.tile_pool(name="consts", bufs=1))
        persist = stack.enter_context(tc.tile_pool(name="persist", bufs=1))

        c16 = consts.tile([128, 2, 128], BF16, tag="c16")  # tri01 | ones
        tri01 = c16[:, 0, :]
        ones_sb = c16[:, 1, :]
        cf = consts.tile([128, 2 * NC + 2 * C], F32, tag="cf")  # bqt | bb
        bqt_sb = cf[:, 0:2 * NC]
        bb_sb = cf[:, 2 * NC:]

        qt = persist.tile([128, NC, T], BF16, tag="qt")
        kt_ = persist.tile([128, NC, T], BF16, tag="kt")
        vp = persist.tile([128, NT, H, D + 1], BF16, tag="vp")

        p1w = stack.enter_context(tc.tile_pool(name="p1w", bufs=1))
        x1t = p1w.tile([128, NC, T], BF16, tag="x1t")
        wq_sb = p1w.tile([128, NC, 3 * C], BF16, tag="wq")

        p23 = stack.enter_context(tc.tile_pool(name="p23", bufs=1))
        attnT = p23.tile([128, NC, T], BF16, tag="attnT")
        wpp = p23.tile([128, NC, C], BF16, tag="wpp")

        # ---- warmup: keep the PE busy + p-state ramping during DMA lead-in
        warm = consts.tile([128, 128], BF16, tag="warm")
        with tc.tile_pool(name="ps0", bufs=1, space="PSUM") as ps0:
            nc.vector.memzero(warm[:])
            wps = ps0.tile([128, 128], F32, tag="wps")
            for _ in range(N_WARMUP):
                nc.tensor.matmul(wps[:], warm[:], warm[:], start=True, stop=True)

        # ---- DMA issue order (SP sequencer ~565 ns per dma_start).
        # wqkv16 is host-permuted into consumption order: the kernel's
        # 128-col block j of wq_sb holds original feature block WPERM[j].
        def dma_wcols(w0, w1):
            nc.sync.dma_start(
                wq_sb[:, :, w0:w1],
                io["wqkv16"][:, w0:w1].rearrange("(k p) w -> p k w", p=128))

        nc.sync.dma_start(x1t[:, 0, :], io["xT16"][0:128, :])
        nc.sync.dma_start(wq_sb[:, 0, 0:256], io["wqkv16"][0:128, 0:256])
        nc.sync.dma_start(
            x1t[:, 1:3, :],
            io["xT16"][128:384, :].rearrange("(k p) t -> p k t", p=128))
        nc.sync.dma_start(          # Q0 | K0 for kt 1..5
            wq_sb[:, 1:6, 0:256],
            io["wqkv16"][128:768, 0:256].rearrange("(k p) w -> p k w", p=128))
        nc.sync.dma_start(
            x1t[:, 3:6, :],
            io["xT16"][384:768, :].rearrange("(k p) t -> p k t", p=128))
        nc.sync.dma_start(c16[:], io["consts16"].rearrange("p (g w) -> p g w", g=2))
        nc.sync.dma_start(cf[:], io["constsf"])
        dma_wcols(256, 512)          # Q1 | K1
        dma_wcols(512, 1280)         # V
        dma_wcols(1280, 1792)        # Q2 K2 Q3 K3
        dma_wcols(1792, 2304)        # Q4 K4 Q5 K5
        nc.sync.dma_start(wpp[:],
                          io["wp16"].rearrange("(k p) w -> p k w", p=128))

        # ones column 0 of vp ([1|V] puts the softmax denominator on row 0)
        nc.vector.tensor_copy(
            vp[:, :, :, 0],
            ones_sb[:, 0:NT * H].rearrange("p (t h) -> p t h", h=H))

        # PSUM budget (8 banks of 512 f32): psv 4 (chains+V+O share) + ps_s 2x2
        stack12 = stack.enter_context(ExitStack())
        psv = stack12.enter_context(tc.tile_pool(name="psv", bufs=4, space="PSUM"))
        ps2a = stack12.enter_context(tc.tile_pool(name="ps2a", bufs=2, space="PSUM"))
        p2 = stack12.enter_context(tc.tile_pool(name="p2", bufs=3))
        p2o = stack12.enter_context(tc.tile_pool(name="p2o", bufs=5))

        # wq_sb 128-col block position of original feature block m
        # (host layout: Q0 K0 Q1 K1 V0..V5 Q2 K2 Q3 K3 Q4 K4 Q5 K5)
        WPOS = {0: 0, 6: 1, 1: 2, 7: 3, 2: 10, 8: 11, 3: 12, 9: 13,
                4: 14, 10: 15, 5: 16, 11: 17}

        def emit_chain(m):
            """One 128-col feature tile of Q^T (m<NC) or K^T (m>=NC)."""
            dest = qt if m < NC else kt_
            mm = m % NC
            wc = WPOS[m] * 128
            for (a, b) in _chunks_512(0, T):
                ps = psv.tile([128, 512], F32, tag="psv")
                for kt in range(NC):
                    nc.tensor.matmul(
                        ps[:, 0:b - a], wq_sb[:, kt, wc:wc + 128],
                        x1t[:, kt, a:b], start=(kt == 0), stop=(kt == NC - 1))
                nc.vector.tensor_scalar_add(dest[:, mm, a:b], ps[:, 0:b - a],
                                            bqt_sb[:, m:m + 1])

        def emit_V():
            for t in range(NT):
                for (n0, n1) in _chunks_512(0, C):
                    ps = psv.tile([128, 512], F32, tag="psv")
                    for kt in range(NC):
                        nc.tensor.matmul(
                            ps[:, 0:n1 - n0], x1t[:, kt, t * 128:(t + 1) * 128],
                            wq_sb[:, kt, 512 + n0:512 + n1],
                            start=(kt == 0), stop=(kt == NC - 1))
                    h0, h1 = n0 // D, n1 // D
                    nc.vector.tensor_tensor(
                        vp[:, t, h0:h1, 1:D + 1],
                        ps[:, 0:n1 - n0].rearrange("p (h d) -> p h d", d=D),
                        bb_sb[:, n0:n1].rearrange("p (h d) -> p h d", d=D),
                        OP.add)

        pt_tiles = {}

        # strips 5/6/7 pack into one 2-bank PSUM tile (each matmul write
        # stays inside a bank: s5@0 w384, s7@384 w128, s6@512 w256) so one
        # exp covers all three - 6 ACT ops/head instead of 8
        FOFF = {5: 0, 6: 512, 7: 384}

        def emit_S(h):
            p0 = 64 * (h % 2)
            ft = h // 2
            # per-strip tiles: fine-grained deps
            pts = [p2.tile([128, T - 128 * kt], BF16, tag=f"pt{kt}",
                           name=f"pt{h}_{kt}") for kt in range(5)]
            ptf = p2.tile([128, 768], BF16, tag="pt567", name=f"ptf{h}")
            pt_tiles[h] = (pts, ptf)
            for kt in range(5):
                base = (kt * 128 // 512) * 512
                ps_s = ps2a.tile([128, T - base], F32, tag="ps_s")
                for (a, b) in _chunks_512(kt * 128, T):
                    nc.tensor.matmul(
                        ps_s[:, a - base:b - base],
                        kt_[p0:p0 + 64, ft, kt * 128:(kt + 1) * 128],
                        qt[p0:p0 + 64, ft, a:b],
                        start=True, stop=True)
                nc.scalar.activation(
                    pts[kt][:, :],
                    ps_s[:, kt * 128 - base:T - base],
                    AF.Exp, bias=0.0, scale=1.0 / np.sqrt(D))
                # causal mask of the diagonal block: multiply by the 0/1
                # lower-triangle tile (DVE 4x bf16 mode) instead of a PE
                # mask matmul
                nc.vector.tensor_tensor(
                    pts[kt][:, 0:128], pts[kt][:, 0:128],
                    tri01[:, :], OP.mult)
            psf = ps2a.tile([128, 768], F32, tag="ps_s", name=f"psf{h}")
            for kt in (5, 6, 7):
                o = FOFF[kt]
                nc.tensor.matmul(
                    psf[:, o:o + T - kt * 128],
                    kt_[p0:p0 + 64, ft, kt * 128:(kt + 1) * 128],
                    qt[p0:p0 + 64, ft, kt * 128:T],
                    start=True, stop=True)
            nc.scalar.activation(ptf[:, :], psf[:, :],
                                 AF.Exp, bias=0.0, scale=1.0 / np.sqrt(D))
            for kt in (5, 6, 7):
                nc.vector.tensor_tensor(
                    ptf[:, FOFF[kt]:FOFF[kt] + 128],
                    ptf[:, FOFF[kt]:FOFF[kt] + 128], tri01[:, :], OP.mult)

        def emit_O(h, rev=False):
            p0 = 64 * (h % 2)
            ft = h // 2
            pts, ptf = pt_tiles.pop(h)
            qchunks = _chunks_512(0, T)
            if rev:
                qchunks = qchunks[::-1]
            tail = []
            for ci, (q0, q1) in enumerate(qchunks):
                kt_max = q1 // 128
                w = q1 - q0
                ps_o = psv.tile([65, 512], F32, tag="psv")
                for kt in range(kt_max):
                    a = max(q0, kt * 128)
                    if kt < 5:
                        rhs = pts[kt][:, a - kt * 128:q1 - kt * 128]
                    else:
                        o = FOFF[kt]
                        rhs = ptf[:, o + a - kt * 128:o + q1 - kt * 128]
                    nc.tensor.matmul(
                        ps_o[:, a - q0:q1 - q0],
                        vp[:, kt, h, :], rhs,
                        start=(kt == 0), stop=(kt == kt_max - 1))
                # copy to SBUF right away: frees the PSUM bank in ~0.7 us
                # instead of holding it through the whole normalize chain
                o_sb = p2o.tile([65, 512], F32, tag="o_sb")
                if h % 2:
                    nc.vector.tensor_copy(o_sb[:, 0:w], ps_o[:, 0:w])
                else:
                    nc.scalar.copy(o_sb[:, 0:w], ps_o[:, 0:w])  # ACT has slack
                # [1|V] puts the denominator on row 0: partition_broadcast
                # reads it directly
                dn = p2o.tile([1, 512], F32, tag="dn")
                bc = p2o.tile([128, 512], F32, tag="bc")
                nc.vector.reciprocal(dn[0:1, 0:w], o_sb[0:1, 0:w])
                nc.gpsimd.partition_broadcast(bc[:, 0:w], dn[0:1, 0:w])
                o_n = p2o.tile([65, 512], BF16, tag="o_n")
                tail.append((o_sb, bc, o_n, q0, q1, w))
                if ci == 0:
                    continue
                _flush_norm(tail.pop(0), ft, p0)
            for item in tail:
                _flush_norm(item, ft, p0)

        def _flush_norm(item, ft, p0):
            o_sb, bc, o_n, q0, q1, w = item
            # row 0 computes d*(1/d) - harmless; DVE partition base must be
            # 32-aligned so the O rows (1..65) can't be addressed directly
            nc.vector.tensor_tensor(o_n[0:65, 0:w], o_sb[0:65, 0:w],
                                    bc[0:65, 0:w], OP.mult)
            # DVE lanes are partition-locked; route the normalized O to its
            # head's partition slot via a small SBUF->SBUF DMA
            nc.sync.dma_start(attnT[p0:p0 + 64, ft, q0:q1], o_n[1:65, 0:w])

        # ---- master schedule -------------------------------------------
        emit_chain(0)      # Q feature tile 0
        emit_chain(NC)     # K feature tile 0
        emit_S(1)
        emit_S(0)
        emit_chain(1)
        emit_chain(NC + 1)
        emit_V()
        emit_O(1)
        emit_O(0)
        def warm_fill(n):
            # keep the PE busy (and its p-state clock hot) through a known
            # stall; the tiles' only consumer is the PE itself
            for _ in range(n):
                wt = psv.tile([128, 128], F32, tag="psv", name="wfill")
                nc.tensor.matmul(wt[:, 0:128], warm[:], warm[:],
                                 start=True, stop=True)

        for j in range(1, H // 2):
            emit_S(2 * j + 1)
            emit_S(2 * j)
            if j + 1 < H // 2:
                # fill the PE while exp(2j+1) drains on ACT; PSUM banks free
                # fast (the copy is the only PSUM reader in the norm chain)
                emit_chain(j + 1)
                emit_chain(NC + j + 1)
            else:
                warm_fill(20)  # no chains left: cover the exp(11) wait
            emit_O(2 * j + 1)
            emit_O(2 * j, rev=(j == H // 2 - 1))

        stack12.close()  # free phase-1/2 PSUM banks for the projection

        # ---------------- projection ----------------
        with tc.tile_pool(name="p3", bufs=3) as p3, \
             tc.tile_pool(name="ps3", bufs=2, space="PSUM") as ps3:
            # last head finished queries 512.. first -> those t-tiles first
            for t in [4, 5, 6, 7, 0, 1, 2, 3]:
                for (n0, n1) in _chunks_512(0, C):
                    ps_y = ps3.tile([128, 512], F32, tag="ps_y")
                    for kt in range(NC):
                        nc.tensor.matmul(
                            ps_y[:, 0:n1 - n0],
                            attnT[:, kt, t * 128:(t + 1) * 128],
                            wpp[:, kt, n0:n1],
                            start=(kt == 0), stop=(kt == NC - 1))
                    y_sb = p3.tile([128, 512], F32, tag="y_sb")
                    nc.vector.tensor_tensor(y_sb[:, 0:n1 - n0],
                                            ps_y[:, 0:n1 - n0],
                                            bb_sb[:, C + n0:C + n1], OP.add)
                    nc.sync.dma_start(io["y"][t * 128:(t + 1) * 128, n0:n1],
                                      y_sb[:, 0:n1 - n0])


def declare_io(nc):
    return {
        "xT16": nc.dram_tensor("xT16", [C, T], BF16, kind="ExternalInput").ap(),
        "wqkv16": nc.dram_tensor("wqkv16", [C, 3 * C], BF16,
                                 kind="ExternalInput").ap(),
        "wp16": nc.dram_tensor("wp16", [C, C], BF16, kind="ExternalInput").ap(),
        "consts16": nc.dram_tensor("consts16", [128, 2 * 128], BF16,
                                   kind="ExternalInput").ap(),
        "constsf": nc.dram_tensor("constsf", [128, 2 * NC + 2 * C], F32,
                                  kind="ExternalInput").ap(),
        "y": nc.dram_tensor("y", [T, C], F32, kind="ExternalOutput").ap(),
    }


def build_nc():
    from concourse import bacc
    import concourse.tile as tile
    nc = bacc.Bacc("TRN2", target_bir_lowering=False, debug=False,
                   enable_asserts=True, num_devices=N_CORES)
    io = declare_io(nc)
    with tile.TileContext(nc) as tc:
        _emit_attention(tc, io)
    nc.compile()
    return nc


def host_consts():
    import ml_dtypes
    # tri01[key p, query c] = 1 where c >= p (keep), 0 above the diagonal
    tri01 = np.triu(np.ones((128, 128), dtype=np.float32))
    consts16 = np.concatenate([
        tri01.astype(ml_dtypes.bfloat16),
        np.ones((128, 128), dtype=ml_dtypes.bfloat16),
    ], axis=1)
    return {"consts16": np.ascontiguousarray(consts16)}


_NC_CACHE = None


def _get_nc():
    global _NC_CACHE
    if _NC_CACHE is None:
        _NC_CACHE = build_nc()
    return _NC_CACHE


# host wqkv column-block order (orig 128-col block index): Q0 K0 Q1 K1
# V0..V5 Q2 K2 Q3 K3 Q4 K4 Q5 K5 -- matches WPOS in _emit_attention
WPERM = [0, 6, 1, 7, 12, 13, 14, 15, 16, 17, 2, 8, 3, 9, 4, 10, 5, 11]


def make_in_maps(x, c_attn_kernel, c_attn_bias, c_proj_kernel, c_proj_bias):
    import ml_dtypes
    BF = ml_dtypes.bfloat16
    consts = host_consts()
    wq_f32 = np.asarray(c_attn_kernel, dtype=np.float32)
    wqkv16 = np.ascontiguousarray(np.concatenate(
        [wq_f32[:, j * 128:(j + 1) * 128] for j in WPERM], axis=1)).astype(BF)
    bqkv = np.ascontiguousarray(c_attn_bias, dtype=np.float32)
    wp16 = np.ascontiguousarray(c_proj_kernel, dtype=np.float32).astype(BF)
    bp = np.ascontiguousarray(c_proj_bias, dtype=np.float32)
    bqt = np.ascontiguousarray(bqkv[:2 * C].reshape(2 * NC, 128).T)
    bias_bcast = np.tile(np.concatenate([bqkv[2 * C:], bp]), (128, 1))
    constsf = np.ascontiguousarray(
        np.concatenate([bqt, bias_bcast], axis=1), dtype=np.float32)
    in_maps = []
    for bb in range(N_CORES):
        m = {"xT16": np.ascontiguousarray(
                np.asarray(x[bb], np.float32).T).astype(BF),
             "wqkv16": wqkv16, "wp16": wp16, "constsf": constsf}
        m.update(consts)
        in_maps.append(m)
    return in_maps


def kernel(x, c_attn_kernel, c_attn_bias, c_proj_kernel, c_proj_bias):
    from concourse.bass_utils import run_bass_kernel_spmd
    x = np.asarray(x)
    assert x.shape == (B, T, C), x.shape
    nc = _get_nc()
    in_maps = make_in_maps(x, c_attn_kernel, c_attn_bias, c_proj_kernel,
                           c_proj_bias)
    res = run_bass_kernel_spmd(nc, in_maps, core_ids=list(range(N_CORES)))
    y = np.stack([res.results[bb]["y"] for bb in range(N_CORES)]).astype(np.float32)
    return y
